# revision 36
# baseline (speedup 1.0000x reference)
"""Trainium2 Bass kernel for a binarized BasicBlock (BinConv3x3 + scale + sync-BN + residual).

Reference computation (NCHW, N=64, C=256, H=W=28):
    out = BN_train(scale * conv3x3(sign(x), sign(w))) + x

Strategy: data-parallel over batch across 8 NeuronCores (8 images/core).
  - host: binarize weights to fp8 e4m3 DoubleRow lhsT tiles, fold gamma/scale/beta
  - device per core:
      sign(x) -> zero-padded fp8 tiles [128cin, 2cib, 30, 30] per image (ScalarE)
      conv3x3 = 9 shifted fp8 DoubleRow matmuls (K=256 per matmul) accumulated
      in PSUM; 2-4 PSUM tiles share each weight load so the PE streams gapless
      PSUM evacuation on ScalarE with row-accumulate -> per-channel sum(z);
      sum(z^2) via VectorE square+reduce
      2KB AllGather of the partial sums across the 8 cores + local reduce
      (exact sync-BN; sums of +-1 dot products are exact integers in fp32)
      per-channel A,B finalization; apply out = A*z + B + x on ScalarE/VectorE
"""

import os
import sys

sys.path.insert(0, "/opt/trn_rl_repo")

import numpy as np
import ml_dtypes

import concourse.mybir as mybir
import concourse.tile as tile
from concourse import bacc
from concourse.bass_utils import run_bass_kernel_spmd

AF = mybir.ActivationFunctionType
ALU = mybir.AluOpType

N_CORES = 8
N_PER_CORE = 8          # images per core
C = 256                 # channels
CB = 2                  # channel blocks of 128
P = 128                 # partitions
H = W = 28
HW = H * W              # 784
HP = WP = 30            # padded spatial
HALF = 14               # output rows per matmul group
NFREE = HALF * W        # 392 free elems per matmul
BN_EPS = 1e-5
N_TOTAL_ELEMS = 64 * HW  # BN normalizer: N*H*W over the full batch

_CACHED = None


def _build_nc():
    nc = bacc.Bacc("TRN2", target_bir_lowering=False, debug=False,
                   num_devices=N_CORES)

    x_dram = nc.dram_tensor("x", [N_PER_CORE, CB, P, HW], mybir.dt.float32,
                            kind="ExternalInput")
    wb_dram = nc.dram_tensor("wb", [P, CB * 9, CB, P], mybir.dt.float8e4,
                             kind="ExternalInput")
    pp_dram = nc.dram_tensor("pp", [P, CB, 3], mybir.dt.float32,
                             kind="ExternalInput")
    out_dram = nc.dram_tensor("out", [N_PER_CORE, CB, P, HW], mybir.dt.float32,
                              kind="ExternalOutput")

    with tile.TileContext(nc) as tc:
        with (
            tc.tile_pool(name="const", bufs=1) as cpool,
            tc.tile_pool(name="xin", bufs=1) as xpool,
            tc.tile_pool(name="spad", bufs=1) as spool,
            tc.tile_pool(name="z", bufs=1) as zpool,
            tc.tile_pool(name="sq", bufs=2) as sqpool,
            tc.tile_pool(name="small", bufs=1) as mpool,
            tc.tile_pool(name="gather", bufs=1) as gpool,
            tc.tile_pool(name="psum", bufs=2, space="PSUM") as psum,
            tc.tile_pool(name="dram", bufs=1, space="DRAM") as dram,
        ):
            # image 0's x goes first on the DMA queue — it gates the first
            # sign and hence the conv start; the weights aren't needed until
            # the first matmul, well after.
            sts = []
            for n in range(N_PER_CORE):
                st = spool.tile([P, CB, HP, WP], mybir.dt.float8e4,
                                name=f"spad{n}", tag=f"spad{n}")
                sts.append(st)
            xcb = [xpool.tile([P, N_PER_CORE, HW], mybir.dt.float32,
                              name=f"xcb{cb}", tag=f"xcb{cb}")
                   for cb in range(CB)]
            # warm tile memset leads the vector queue: the HAM warmup matmuls
            # are gated only by it, and the PE should ramp as early as
            # possible
            warm = cpool.tile([P, 512], mybir.dt.float8e4)
            nc.vector.memset(warm[:], 1.0)
            for cb in range(CB):
                nc.vector.memset(sts[0][:, cb], 0.0)
                nc.sync.dma_start(xcb[cb][:, 0, :], x_dram[0, cb])

            wt = cpool.tile([P, CB * 9, CB, P], mybir.dt.float8e4)
            nc.sync.dma_start(wt[:], wb_dram[:])

            # Preload the gpsimd extended-inst ucode library during the DMA
            # lead-in: a sem-only broadcast-to-self PREP forces the
            # UNLOAD/LOAD_LIB pair here instead of on the critical path of
            # the stats exchange (~5.5us of library load from HBM). The desc
            # only sits in the SWDGE ring — it is fired later by the
            # exchange's trigger (count=2), once the fabric is known-safe.
            dsem = nc.alloc_semaphore("libwarm_rsem")
            dlsem = nc.alloc_semaphore("libwarm_lsem")
            psem = nc.alloc_semaphore("stats_psem")
            nc.gpsimd.remote_sem_update_broadcast(
                remote_sem=dsem, local_sem=dlsem,
                rdests=[(0, 0)] + [None] * (N_CORES - 1))

            # Pre-generate the stats-exchange broadcast descriptor NOW (the
            # Switch dispatch, crit-section entry, and ~0.9us descgen all
            # happen during the conv, off the critical path). The descriptor
            # defers its read of cc_sb to trigger time; only the
            # barrier-wait + trigger_dma(count=2) remain at stats time.
            rsem = nc.alloc_semaphore("stats_rsem")
            lsem = nc.alloc_semaphore("stats_lsem")
            cc_sb = mpool.tile([P, 4], mybir.dt.float32)
            gt = gpool.tile([P, N_CORES, 4], mybir.dt.float32)
            rank = nc.gpsimd.partition_id()
            rdall = [(0, k) for k in range(N_CORES)]
            with tc.tile_critical(name="stats_xchg", no_gpsimd_drain=True):
                for r in nc.gpsimd.Switch(rank, N_CORES):
                    nc.gpsimd.remote_dma_broadcast(
                        gt[:, r], cc_sb[:], remote_sem=rsem, local_sem=lsem,
                        rdests=rdall).then_inc(psem, 1)
            # Detach the preps from the (much later) trigger so the
            # scheduling sim doesn't make the trigger wait on the 7
            # never-executed cases; the cc_sb ordering the links would have
            # provided is re-established by the gate reg_loads below.
            nc.gpsimd._pending_untriggered_insts[0].clear()

            # Force the Sign ACT LUT load at kernel start (otherwise walrus
            # schedules it right before the first real sign, gating the PE).
            dummy_sg = cpool.tile([P, 1], mybir.dt.float8e4)
            nc.scalar.activation(dummy_sg[:], nc.const_aps.tensor(0.0, (P, 1)),
                                 AF.Sign)

            # HAM warm-up: dummy matmuls during the otherwise-idle DMA
            # lead-in, so the PE is already un-throttled when the real conv
            # stream starts.
            for _i in range(9):
                wps = psum.tile([P, 4, 512], mybir.dt.float32, tag="ps")
                nc.tensor.matmul(wps[:, 0], warm[:, 0:P], warm[:],
                                 start=True, stop=True)

            # sign(x) into the interior of the zero-padded per-image tiles
            # [128, cib, 30, 30] (fp8 for DoubleRow); x lives in one tile per
            # channel-block so the residual adds can process two images per
            # DVE op (Tile's dependency tracking is range-based, so per-image
            # DMA slices still gate signs per image)
            pp = None
            for n in range(N_PER_CORE):
                for cb in range(CB):
                    if n > 0:
                        nc.vector.memset(sts[n][:, cb], 0.0)
                        nc.sync.dma_start(xcb[cb][:, n, :], x_dram[n, cb])
                    nc.scalar.activation(sts[n][:, cb, 1:29, 1:29],
                                         xcb[cb][:, n, :], AF.Sign)
                if n == 0:
                    pp = cpool.tile([P, CB, 3], mybir.dt.float32)
                    nc.sync.dma_start(pp[:], pp_dram[:])

            # conv output, raw (unscaled) integer-valued sums
            z = zpool.tile([P, CB, N_PER_CORE, HW], mybir.dt.float32)
            # per-group partials (one column per (group, cob) evacuation)
            GROUPS = [[(0, 0), (0, 1)],
                      [(1, 0), (1, 1)],
                      [(2, 0), (2, 1), (3, 0), (3, 1)],
                      [(4, 0), (4, 1), (5, 0), (5, 1)],
                      [(6, 0), (6, 1)],
                      [(7, 0), (7, 1)]]
            NG = len(GROUPS)
            s1c = mpool.tile([P, CB, NG], mybir.dt.float32)
            s2c = mpool.tile([P, CB, NG], mybir.dt.float32)

            # Conv via fp8 DoubleRow: each matmul contracts both cin-blocks
            # (K=256) at once; the group's units accumulate in adjacent PSUM
            # banks of ONE 4-bank tile per weight load so each lhsT is
            # reused and the PE streams. Evacuation is one ACT pass over the
            # whole group (strided across banks) with a single accumulator
            # read for s1, and one DVE square+reduce pair for s2 — per-unit
            # accumulator reads were ~290ns each of pure ACT overhead.
            for g, units in enumerate(GROUPS):
                for cob in range(CB):
                    U = len(units)
                    pss = psum.tile([P, 4, 512], mybir.dt.float32,
                                    name=f"ps_{g}_{cob}", tag="ps")
                    for dh in range(3):
                        for dw in range(3):
                            w_ap = wt[:, cob * 9 + dh * 3 + dw, :, :]
                            first = (dh == 0 and dw == 0)
                            last = (dh == 2 and dw == 2)
                            for j, (n, half) in enumerate(units):
                                h0 = half * HALF
                                nc.tensor.matmul(
                                    pss[:, j, 0:NFREE],
                                    w_ap,
                                    sts[n][:, :, h0 + dh:h0 + dh + HALF,
                                           dw:dw + W],
                                    start=first,
                                    stop=last,
                                    perf_mode=mybir.MatmulPerfMode.DoubleRow,
                                )
                    n0 = units[0][0]
                    nimg = U // 2
                    zsl = z[:, cob, n0:n0 + nimg, :].rearrange(
                        "p n (h f) -> p (n h) f", f=NFREE)
                    nc.scalar.activation(
                        zsl, pss[:, 0:U, 0:NFREE],
                        AF.Copy, accum_out=s1c[:, cob, g:g + 1])
                    zf = z[:, cob, n0:n0 + nimg, :].rearrange(
                        "p n f -> p (n f)")
                    sq = sqpool.tile([P, 4 * NFREE], mybir.dt.float32,
                                     tag="sq")
                    nc.vector.tensor_mul(sq[:, 0:U * NFREE], zf, zf)
                    nc.vector.tensor_reduce(
                        s2c[:, cob, g:g + 1], sq[:, 0:U * NFREE],
                        axis=mybir.AxisListType.X, op=ALU.add)

            # local stats -> [128, 4] = [s1_b0, s1_b1, s2_b0, s2_b1]
            nc.vector.tensor_reduce(cc_sb[:, 0:2], s1c[:],
                                    axis=mybir.AxisListType.X, op=ALU.add)
            nc.vector.tensor_reduce(cc_sb[:, 2:4], s2c[:],
                                    axis=mybir.AxisListType.X, op=ALU.add)

            # exact sync-BN via DIY peer exchange: the runtime cc-AllGather
            # costs ~38us (trigger delay + mesh op) for 2KB; instead each
            # core remote-DMAs its [128,4] partial sums straight into every
            # peer's SBUF, landing at slot = sender's rank on every receiver
            # (one broadcast to all 8 same-device peers, self via fabric
            # loopback; each dest pair-of-lanes delivers remote_sem += 2, so
            # all-arrived reads rsem == 16). The descriptor was pre-generated
            # above; only the trigger (firing the lib-warm desc + the real
            # one) sits here. The runtime-only waits (peer-entry barrier
            # before the trigger, arrival sem before the reduce) are spliced
            # in AFTER tile scheduling — the single-core scheduling sim
            # can't model remote sem increments and would deadlock on them.
            # gate the trigger on the stats (tile wires vector->gpsimd
            # semaphores through this dummy copy+load chain; the
            # descriptor's deferred cc_sb read is only safe after it)
            gti = mpool.tile([P, 4], mybir.dt.int32)
            nc.vector.tensor_copy(gti[0:1, 0:4], cc_sb[0:1, 0:4])
            gate = nc.gpsimd.alloc_register("xchg_gate")
            nc.gpsimd.reg_load(gate, gti[0:1, 0:1])
            tc.no_sync_barrier()  # trigger must stay after the gates
            trig_inss = [nc.gpsimd.trigger_dma(count=2).ins]
            tot = mpool.tile([P, 4], mybir.dt.float32)
            red_ins = nc.vector.tensor_reduce(
                tot[:], gt[:].rearrange("p r c -> p c r"),
                axis=mybir.AxisListType.X, op=ALU.add).ins

            # per-channel finalization:
            #   mu_z = S1/M ; var_z = S2/M - mu_z^2 ; var_y = scale^2*var_z
            #   A = gamma*scale/sqrt(var_y+eps) ; B = beta - A*mu_z
            inv = 1.0 / N_TOTAL_ELEMS
            mm4 = mpool.tile([P, 4], mybir.dt.float32)
            m2 = mpool.tile([P, CB], mybir.dt.float32)
            varz = mpool.tile([P, CB], mybir.dt.float32)
            vary = mpool.tile([P, CB], mybir.dt.float32)
            stdv = mpool.tile([P, CB], mybir.dt.float32)
            rstd = mpool.tile([P, CB], mybir.dt.float32)
            A = mpool.tile([P, CB], mybir.dt.float32)
            t0 = mpool.tile([P, CB], mybir.dt.float32)
            B = mpool.tile([P, CB], mybir.dt.float32)

            nc.vector.tensor_scalar_mul(mm4[:], tot[:], inv)
            mu = mm4[:, 0:2]
            ez2 = mm4[:, 2:4]
            nc.vector.tensor_mul(m2[:], mu, mu)
            nc.vector.tensor_sub(varz[:], ez2, m2[:])
            nc.vector.tensor_mul(vary[:], varz[:], pp[:, :, 0])
            nc.vector.tensor_scalar_add(vary[:], vary[:], BN_EPS)
            nc.scalar.activation(stdv[:], vary[:], AF.Sqrt)
            nc.vector.reciprocal(rstd[:], stdv[:])
            nc.vector.tensor_mul(A[:], rstd[:], pp[:, :, 1])
            nc.vector.tensor_mul(t0[:], A[:], mu)
            nc.vector.tensor_sub(B[:], pp[:, :, 2], t0[:])

            # apply: out = A*z + B + x, then DMA out. The ACT scale-bias pass
            # covers up to two adjacent images per op (same per-channel A/B,
            # z is contiguous), pipelined with DVE residual adds; the first
            # two chunks are single-image so the out-DMA stream starts as
            # early as possible, and each DMA moves one image. GpSimd stays
            # off SBUF ports — its elementwise ops contend with DVE for the
            # shared port pair.
            chunks = [(0, 1), (1, 1), (2, 2), (4, 2), (6, 2)]
            for n0, cnt in chunks:
                for cb in range(CB):
                    zs2 = z[:, cb, n0:n0 + cnt, :]
                    nc.scalar.activation(zs2, zs2, AF.Identity,
                                         scale=A[:, cb:cb + 1],
                                         bias=B[:, cb:cb + 1])
                    nc.vector.tensor_add(zs2, zs2, xcb[cb][:, n0:n0 + cnt, :])
                    for j in range(cnt):
                        nc.sync.dma_start(out_dram[n0 + j, cb],
                                          z[:, cb, n0 + j, :])

    # Inject the runtime-only cross-core waits now that tile scheduling is
    # done (same post-scheduling list-splice the framework itself uses for
    # the bir-kernel-barrier prelude). Emission appends to the block; move
    # each wait in front of its consumer so the engine streams gate on it.
    def _find(ins):
        for b in nc.main_func.blocks:
            for i, cand in enumerate(b.instructions):
                if cand is ins:
                    return b, i
        raise RuntimeError(f"instruction {ins.name} not found")

    moves = [(nc.vector.wait_ge(rsem, 2 * N_CORES).ins, red_ins)]
    for trig in trig_inss:
        moves.append(
            (nc.gpsimd.bir_kernel_barrier_wait([list(range(N_CORES))]).ins,
             trig))
        # descgen-committed guard (runtime-trivially-true: the prep ran
        # during the conv; sim can't model it because the executed Switch
        # case is data-dependent)
        moves.append((nc.gpsimd.wait_ge(psem, 1).ins, trig))
    for w, target in moves:
        wb, wi = _find(w)
        del wb.instructions[wi]
        tb, ti = _find(target)
        tb.instructions.insert(ti, w)

    nc.compile()
    return nc


def _prep_shared(w, scale, gamma, beta):
    w = np.asarray(w, dtype=np.float32)
    scale = np.asarray(scale, dtype=np.float32).reshape(C)
    gamma = np.asarray(gamma, dtype=np.float32).reshape(C)
    beta = np.asarray(beta, dtype=np.float32).reshape(C)

    # DoubleRow lhsT[k, idx=(cob,dh,dw), r, m] = sign(w)[cob*128+m, r*128+k, dh, dw]
    # stored [k][idx][r][m] (contiguous per partition k) as fp8 e4m3.
    wsign = np.sign(w).astype(ml_dtypes.float8_e4m3)
    arr = wsign.reshape(CB, P, CB, P, 3, 3).transpose(3, 0, 4, 5, 2, 1)
    wb = np.ascontiguousarray(arr.reshape(P, CB * 9, CB, P))

    pp = np.empty((P, CB, 3), dtype=np.float32)
    for cb in range(CB):
        ch = slice(cb * P, (cb + 1) * P)
        pp[:, cb, 0] = scale[ch] * scale[ch]
        pp[:, cb, 1] = gamma[ch] * scale[ch]
        pp[:, cb, 2] = beta[ch]
    return wb, pp


def kernel(x, w, scale, gamma, beta):
    global _CACHED
    if _CACHED is None:
        _CACHED = _build_nc()
    nc = _CACHED

    x = np.asarray(x, dtype=np.float32)
    wb, pp = _prep_shared(w, scale, gamma, beta)

    in_maps = []
    for i in range(N_CORES):
        xs = x[i * N_PER_CORE:(i + 1) * N_PER_CORE]
        xs = np.ascontiguousarray(xs.reshape(N_PER_CORE, CB, P, HW))
        in_maps.append({"x": xs, "wb": wb, "pp": pp})

    trace = bool(int(os.environ.get("KERNEL_TRACE", "0")))
    kw = {}
    tdir = os.environ.get("KERNEL_TRACE_DIR")
    if trace and tdir:
        global _NCALL
        _NCALL = globals().get("_NCALL", 0) + 1
        tdir = os.path.join(tdir, f"call{_NCALL}")
        os.makedirs(tdir, exist_ok=True)
        kw["tmpdir"] = tdir
    res = run_bass_kernel_spmd(nc, in_maps, core_ids=list(range(N_CORES)),
                               trace=trace, **kw)
    if trace:
        globals()["LAST_EXEC_NS"] = res.exec_time_ns
        globals()["LAST_RESULTS"] = res

    out = np.empty((64, C, H, W), dtype=np.float32)
    for i in range(N_CORES):
        o = res.results[i]["out"].reshape(N_PER_CORE, C, H, W)
        out[i * N_PER_CORE:(i + 1) * N_PER_CORE] = o
    return out

